# revision 2
# baseline (speedup 1.0000x reference)
"""Divergence-free RBF kernel Gram matrix on 8 Trainium2 NeuronCores. v2.

Math: for d=2, with scaled coords x' = x*exp(-ll/2):
  dx = x0_i - y0_j, dy = x1_i - y1_j, r2 = dx^2 + dy^2, e = exp(-r2/2)
  K[2i+0, 2j+0] = e * (1 - dy^2)   = e * p00
  K[2i+0, 2j+1] = K[2i+1, 2j+0] = e * dx*dy = e * p01
  K[2i+1, 2j+1] = e * (1 - dx^2)   = e * p11

v2 design (vs v1): device emits three COMPACT fp16 planes out[c, i, j] =
e*p_c (c in {00, 01, 11}) instead of the interleaved f32 matrix; the host
interleaves rows/cols and upcasts. This cuts HBM writes 2.67x (12.6MB/core)
and halves+ the elementwise work. Each plane (and r2) is low-rank in the
X-basis {1, x0, x1, x0*x1, x0^2, x1^2} (K=6, hi/lo bf16 split stacked to
K=18 for fp32-grade accuracy): PE matmuls produce p01/p00/p11/r2 per
(128 x 512) chunk (PE col-rate 1.2GHz locked -> ~55us/core, the wall),
ACT computes e = exp(-r2/2) into fp16, and the 3 e*p multiplies per chunk
are split DVE-direct-from-PSUM vs GPSIMD-from-fp16-SBUF (after an ACT
copy) per a tunable class pattern so DVE/ACT/GP all stay under the PE wall.

Sharding: rows of X (n axis) split across 8 cores, 512 each; no comms.
"""

import numpy as np
import ml_dtypes

N = 4096          # X rows
M = 4096          # Y rows
D = 2
NCORES = 8
NPC = N // NCORES  # 512 X rows per core
IB = 128           # i-block = partition count
NIB = NPC // IB    # 4 i-blocks per core
JC = 512           # j-chunk per PSUM plane tile
NJC = M // JC      # 8 chunks per i-block
KST = 18           # stacked contraction dim (3 x 6 basis rows)

_cache = {}


def _hi_lo(a):
    bf = ml_dtypes.bfloat16
    hi = a.astype(bf)
    lo = (a - hi.astype(np.float64)).astype(bf)
    return hi, lo


def _prepare_inputs(X, Y, log_length_scale):
    s = float(np.exp(-0.5 * np.float64(np.asarray(log_length_scale).reshape(-1)[0])))
    xs = np.asarray(X, dtype=np.float64).reshape(N, D) * s
    ys = np.asarray(Y, dtype=np.float64).reshape(M, D) * s
    x0, x1 = xs[:, 0], xs[:, 1]
    y0, y1 = ys[:, 0], ys[:, 1]
    one_n, zero_m, one_m = np.ones(N), np.zeros(M), np.ones(M)

    # X-side basis [6, N]: rows {1, x0, x1, x0*x1, x0^2, x1^2}
    L = np.stack([one_n, x0, x1, x0 * x1, x0 ** 2, x1 ** 2])

    # Y-side coefficient columns [6, M] per plane; device plane order is
    # (p01, p00, p11, r2) so the GPSIMD-bound planes p00/p11 are contiguous
    c_01 = np.stack([y0 * y1, -y1, -y0, one_m, zero_m, zero_m])
    c_00 = np.stack([1 - y1 ** 2, zero_m, 2 * y1, zero_m, zero_m, -one_m])
    c_11 = np.stack([1 - y0 ** 2, 2 * y0, zero_m, zero_m, -one_m, zero_m])
    c_r2 = np.stack([y0 ** 2 + y1 ** 2, -2 * y0, -2 * y1, zero_m, one_m, one_m])

    R = np.concatenate([c_01, c_00, c_11, c_r2], axis=1)  # (6, 4M) plane-major

    Lh, Ll = _hi_lo(L)
    Lst = np.ascontiguousarray(np.concatenate([Lh, Ll, Lh], axis=0))  # (18, N)
    Rh, Rl = _hi_lo(R)
    Rst = np.ascontiguousarray(np.concatenate([Rh, Rh, Rl], axis=0))  # (18, 4M)
    return Lst, Rst


def _build_module(bass_cls=None, **bass_kw):
    from concourse import bacc, mybir
    import concourse.tile as tile

    bf16 = mybir.dt.bfloat16
    f16 = mybir.dt.float16
    f32 = mybir.dt.float32
    Exp = mybir.ActivationFunctionType.Exp

    if bass_cls is None:
        bass_cls = bacc.Bacc
    nc = bass_cls("TRN2", target_bir_lowering=False, debug=False,
                  enable_asserts=False, **bass_kw)
    lhsT_d = nc.dram_tensor("lhsT", [KST, NPC], bf16, kind="ExternalInput")
    r_d = nc.dram_tensor("r_pl", [KST, 4 * M], bf16, kind="ExternalInput")
    # out planes: row-blocks (p00, p01, p11), each [NPC, M] fp16
    out_d = nc.dram_tensor("out", [3 * NPC, M], f16, kind="ExternalOutput")

    # psum plane slot (p01, p00, p11) -> output plane row-block
    OUT_SLOT = (1, 0, 2)
    JH = M // 2  # j-half per out tile / DMA

    with tile.TileContext(nc) as tc:
        with (
            tc.tile_pool(name="const", bufs=1) as cpool,
            tc.tile_pool(name="ep", bufs=4) as epool,
            tc.tile_pool(name="mp", bufs=4) as mpool,
            tc.tile_pool(name="outp", bufs=2) as opool,
            tc.tile_pool(name="r2ps", bufs=2, space="PSUM") as r2pool,
            tc.tile_pool(name="plAps", bufs=2, space="PSUM") as plApool,
            tc.tile_pool(name="plBps", bufs=2, space="PSUM") as plBpool,
        ):
            wt = cpool.tile([KST, NPC], bf16)
            nc.sync.dma_start(out=wt[:], in_=lhsT_d[:, :])
            # per-plane, per-j-half input tiles, loaded in first-use order
            # (r2 first) so PE can start as early as possible
            MH = M // 2
            r_sb = [[None, None] for _ in range(4)]
            for h in range(2):
                for p in (3, 0, 1, 2):  # dram plane order: p01, p00, p11, r2
                    rt = cpool.tile([KST, MH], bf16, tag=f"r{p}h{h}",
                                    name=f"r{p}h{h}")
                    nc.sync.dma_start(
                        out=rt[:], in_=r_d[:, p * M + h * MH:p * M + (h + 1) * MH])
                    r_sb[p][h] = rt

            out_v = out_d.ap().rearrange("(c i) j -> c i j", c=3)

            for ib in range(NIB):
                wtb = wt[:, ib * IB:(ib + 1) * IB]
                i0 = ib * IB
                od = out_v[:, i0:i0 + IB, :].rearrange("c p j -> p c j")
                for jq in range(2):
                    out_sb = opool.tile([IB, 3 * JH], f16, tag="out")
                    vout = out_sb[:].rearrange("p (c j) -> p c j", c=3)
                    for hc in range(4):
                        jc = jq * 4 + hc
                        rh, ro = jc // 4, (jc % 4) * JC
                        # r2 in its own psum tile; exp is its only reader
                        r2t = r2pool.tile([IB, JC], f32, tag="r2")
                        nc.tensor.matmul(
                            r2t[:], wtb, r_sb[3][rh][:, ro:ro + JC],
                            start=True, stop=True)
                        et = epool.tile([IB, JC], f16, tag="e")
                        nc.scalar.activation(et[:], r2t[:], Exp, scale=-0.5)
                        # plane psum tiles: A = [p01 | p00] (drained by DVE's
                        # two multiplies), B = p11 (drained by one ACT copy
                        # feeding GPSIMD) -- every consumer runs under the PE
                        # production rate so PE never waits on PSUM
                        plA = plApool.tile([IB, 2 * JC], f32, tag="plA")
                        plB = plBpool.tile([IB, JC], f32, tag="plB")
                        nc.tensor.matmul(
                            plA[:, 0:JC], wtb, r_sb[0][rh][:, ro:ro + JC],
                            start=True, stop=True)
                        nc.tensor.matmul(
                            plA[:, JC:2 * JC], wtb, r_sb[1][rh][:, ro:ro + JC],
                            start=True, stop=True)
                        nc.tensor.matmul(
                            plB[:], wtb, r_sb[2][rh][:, ro:ro + JC],
                            start=True, stop=True)
                        mt = mpool.tile([IB, JC], f16, tag="m")
                        nc.scalar.copy(mt[:], plB[:])
                        esl = et[:]
                        for p in range(2):  # p01 -> plane 1, p00 -> plane 0
                            o = OUT_SLOT[p]
                            nc.vector.tensor_mul(
                                vout[:, o:o + 1, hc * JC:(hc + 1) * JC].squeeze(1),
                                plA[:, p * JC:(p + 1) * JC], esl)
                        nc.gpsimd.tensor_mul(
                            vout[:, 2:3, hc * JC:(hc + 1) * JC].squeeze(1),
                            mt[:], esl)
                    # DMA this half-block ([128, 3, 2048] = 1.5 MB),
                    # partition-major on both sides so descriptors spray
                    # across all 16 SDMA engines
                    nc.sync.dma_start(
                        out=od[:, :, jq * JH:(jq + 1) * JH],
                        in_=out_sb[:].rearrange("p (c j) -> p c j", c=3))
    nc.finalize()
    return nc


def _run(X, Y, log_length_scale, trace=False):
    from concourse.bass_utils import run_bass_kernel_spmd

    Lst, Rst = _prepare_inputs(X, Y, log_length_scale)
    if "nc" not in _cache:
        _cache["nc"] = _build_module()
    nc = _cache["nc"]
    in_maps = [
        {
            "lhsT": np.ascontiguousarray(Lst[:, c * NPC:(c + 1) * NPC]),
            "r_pl": Rst,
        }
        for c in range(NCORES)
    ]
    res = run_bass_kernel_spmd(nc, in_maps, core_ids=list(range(NCORES)),
                               trace=trace)
    # reassemble: per core out [3, 512, 4096] fp16 planes -> (1, 2N, 2M) f32
    planes = np.concatenate(
        [r["out"].reshape(3, NPC, M) for r in res.results], axis=1)  # (3, N, M)
    K = np.empty((N, 2, M, 2), dtype=np.float32)
    K[:, 0, :, 0] = planes[0]
    K[:, 0, :, 1] = planes[1]
    K[:, 1, :, 0] = planes[1]
    K[:, 1, :, 1] = planes[2]
    return K.reshape(1, 2 * N, 2 * M), res


def kernel(X, Y, log_length_scale):
    out, _ = _run(np.asarray(X), np.asarray(Y), np.asarray(log_length_scale))
    return out


# revision 3
# speedup vs baseline: 1.1198x; 1.1198x over previous
"""Divergence-free RBF kernel Gram matrix on 8 Trainium2 NeuronCores. v2.

Math: for d=2, with scaled coords x' = x*exp(-ll/2):
  dx = x0_i - y0_j, dy = x1_i - y1_j, r2 = dx^2 + dy^2, e = exp(-r2/2)
  K[2i+0, 2j+0] = e * (1 - dy^2)   = e * p00
  K[2i+0, 2j+1] = K[2i+1, 2j+0] = e * dx*dy = e * p01
  K[2i+1, 2j+1] = e * (1 - dx^2)   = e * p11

v2 design (vs v1): device emits three COMPACT fp16 planes out[c, i, j] =
e*p_c (c in {00, 01, 11}) instead of the interleaved f32 matrix; the host
interleaves rows/cols and upcasts. This cuts HBM writes 2.67x (12.6MB/core)
and halves+ the elementwise work. Each plane (and r2) is low-rank in the
X-basis {1, x0, x1, x0*x1, x0^2, x1^2} (K=6, hi/lo bf16 split stacked to
K=18 for fp32-grade accuracy): PE matmuls produce p01/p00/p11/r2 per
(128 x 512) chunk (PE col-rate 1.2GHz locked -> ~55us/core, the wall),
ACT computes e = exp(-r2/2) into fp16, and the 3 e*p multiplies per chunk
are split DVE-direct-from-PSUM vs GPSIMD-from-fp16-SBUF (after an ACT
copy) per a tunable class pattern so DVE/ACT/GP all stay under the PE wall.

Sharding: rows of X (n axis) split across 8 cores, 512 each; no comms.
"""

import numpy as np
import ml_dtypes

N = 4096          # X rows
M = 4096          # Y rows
D = 2
NCORES = 8
NPC = N // NCORES  # 512 X rows per core
IB = 128           # i-block = partition count
NIB = NPC // IB    # 4 i-blocks per core
JC = 512           # j-chunk per PSUM plane tile
NJC = M // JC      # 8 chunks per i-block
KST = 18           # stacked contraction dim (3 x 6 basis rows)

_cache = {}


def _hi_lo(a):
    bf = ml_dtypes.bfloat16
    hi = a.astype(bf)
    lo = (a - hi.astype(np.float64)).astype(bf)
    return hi, lo


def _prepare_inputs(X, Y, log_length_scale):
    s = float(np.exp(-0.5 * np.float64(np.asarray(log_length_scale).reshape(-1)[0])))
    xs = np.asarray(X, dtype=np.float64).reshape(N, D) * s
    ys = np.asarray(Y, dtype=np.float64).reshape(M, D) * s
    x0, x1 = xs[:, 0], xs[:, 1]
    y0, y1 = ys[:, 0], ys[:, 1]
    one_n, zero_m, one_m = np.ones(N), np.zeros(M), np.ones(M)

    # X-side basis [6, N]: rows {1, x0, x1, x0*x1, x0^2, x1^2}
    L = np.stack([one_n, x0, x1, x0 * x1, x0 ** 2, x1 ** 2])

    # Y-side coefficient columns [6, M] per plane; device plane order is
    # (p01, p00, p11, r2) so the GPSIMD-bound planes p00/p11 are contiguous
    c_01 = np.stack([y0 * y1, -y1, -y0, one_m, zero_m, zero_m])
    c_00 = np.stack([1 - y1 ** 2, zero_m, 2 * y1, zero_m, zero_m, -one_m])
    c_11 = np.stack([1 - y0 ** 2, 2 * y0, zero_m, zero_m, -one_m, zero_m])
    c_r2 = np.stack([y0 ** 2 + y1 ** 2, -2 * y0, -2 * y1, zero_m, one_m, one_m])

    R = np.concatenate([c_01, c_00, c_11, c_r2], axis=1)  # (6, 4M) plane-major

    Lh, Ll = _hi_lo(L)
    Lst = np.ascontiguousarray(np.concatenate([Lh, Ll, Lh], axis=0))  # (18, N)
    Rh, Rl = _hi_lo(R)
    Rst = np.ascontiguousarray(np.concatenate([Rh, Rh, Rl], axis=0))  # (18, 4M)
    return Lst, Rst


def _build_module(bass_cls=None, **bass_kw):
    from concourse import bacc, mybir
    import concourse.tile as tile

    bf16 = mybir.dt.bfloat16
    f16 = mybir.dt.float16
    f32 = mybir.dt.float32
    Exp = mybir.ActivationFunctionType.Exp

    if bass_cls is None:
        bass_cls = bacc.Bacc
    nc = bass_cls("TRN2", target_bir_lowering=False, debug=False,
                  enable_asserts=False, **bass_kw)
    lhsT_d = nc.dram_tensor("lhsT", [KST, NPC], bf16, kind="ExternalInput")
    r_d = nc.dram_tensor("r_pl", [KST, 4 * M], bf16, kind="ExternalInput")
    # out planes: row-blocks (p00, p01, p11), each [NPC, M] fp16
    out_d = nc.dram_tensor("out", [3 * NPC, M], f16, kind="ExternalOutput")

    # psum plane slot (p01, p00, p11) -> output plane row-block
    OUT_SLOT = (1, 0, 2)
    JH = M // 2  # j-half per out tile / DMA

    with tile.TileContext(nc) as tc:
        with (
            tc.tile_pool(name="const", bufs=1) as cpool,
            tc.tile_pool(name="ep", bufs=4) as epool,
            tc.tile_pool(name="mp", bufs=4) as mpool,
            tc.tile_pool(name="outp", bufs=2) as opool,
            tc.tile_pool(name="r2ps", bufs=2, space="PSUM") as r2pool,
            tc.tile_pool(name="plAps", bufs=2, space="PSUM") as plApool,
            tc.tile_pool(name="plBps", bufs=2, space="PSUM") as plBpool,
        ):
            wt = cpool.tile([KST, NPC], bf16)
            nc.sync.dma_start(out=wt[:], in_=lhsT_d[:, :])
            # per-plane, per-j-quarter input tiles, loaded in first-use order
            # (r2 first) so PE can start as early as possible
            MQ = M // 4
            r_sb = [[None] * 4 for _ in range(4)]
            for h in range(4):
                for p in (3, 0, 1, 2):  # dram plane order: p01, p00, p11, r2
                    rt = cpool.tile([KST, MQ], bf16, tag=f"r{p}h{h}",
                                    name=f"r{p}h{h}")
                    nc.sync.dma_start(
                        out=rt[:], in_=r_d[:, p * M + h * MQ:p * M + (h + 1) * MQ])
                    r_sb[p][h] = rt

            out_v = out_d.ap().rearrange("(c i) j -> c i j", c=3)

            for ib in range(NIB):
                wtb = wt[:, ib * IB:(ib + 1) * IB]
                i0 = ib * IB
                od = out_v[:, i0:i0 + IB, :].rearrange("c p j -> p c j")
                # the very last half-block uses two quarter tiles so its DMA
                # starts halfway through and the drain tail is shorter
                segs = ([(0, 2), (2, 4)] if ib == NIB - 1 else [(0, 4)])
                for jq in range(2):
                  for (h0, h1) in (segs if jq == 1 else [(0, 4)]):
                    segw = (h1 - h0) * JC
                    out_sb = opool.tile([IB, 3 * segw], f16,
                                        tag=f"out{h1 - h0}", name="out_sb")
                    vout = out_sb[:].rearrange("p (c j) -> p c j", c=3)
                    for hc in range(h0, h1):
                        jc = jq * 4 + hc
                        rh, ro = jc // 2, (jc % 2) * JC
                        sc = hc - h0
                        # r2 in its own psum tile; exp is its only reader
                        r2t = r2pool.tile([IB, JC], f32, tag="r2")
                        nc.tensor.matmul(
                            r2t[:], wtb, r_sb[3][rh][:, ro:ro + JC],
                            start=True, stop=True)
                        et = epool.tile([IB, JC], f16, tag="e")
                        nc.scalar.activation(et[:], r2t[:], Exp, scale=-0.5)
                        # plane psum tiles: A = [p01 | p00] (drained by DVE's
                        # two multiplies), B = p11 (drained by one ACT copy
                        # feeding GPSIMD) -- every consumer runs under the PE
                        # production rate so PE never waits on PSUM
                        plA = plApool.tile([IB, 2 * JC], f32, tag="plA")
                        plB = plBpool.tile([IB, JC], f32, tag="plB")
                        nc.tensor.matmul(
                            plA[:, 0:JC], wtb, r_sb[0][rh][:, ro:ro + JC],
                            start=True, stop=True)
                        nc.tensor.matmul(
                            plA[:, JC:2 * JC], wtb, r_sb[1][rh][:, ro:ro + JC],
                            start=True, stop=True)
                        nc.tensor.matmul(
                            plB[:], wtb, r_sb[2][rh][:, ro:ro + JC],
                            start=True, stop=True)
                        mt = mpool.tile([IB, JC], f16, tag="m")
                        nc.scalar.copy(mt[:], plB[:])
                        esl = et[:]
                        for p in range(2):  # p01 -> plane 1, p00 -> plane 0
                            o = OUT_SLOT[p]
                            nc.vector.tensor_mul(
                                vout[:, o:o + 1, sc * JC:(sc + 1) * JC].squeeze(1),
                                plA[:, p * JC:(p + 1) * JC], esl)
                        nc.gpsimd.tensor_mul(
                            vout[:, 2:3, sc * JC:(sc + 1) * JC].squeeze(1),
                            mt[:], esl)
                    # DMA this segment (usually a half-block [128, 3, 2048]
                    # = 1.5 MB), partition-major on both sides so descriptors
                    # spray across all 16 SDMA engines
                    nc.sync.dma_start(
                        out=od[:, :, jq * JH + h0 * JC:jq * JH + h1 * JC],
                        in_=out_sb[:].rearrange("p (c j) -> p c j", c=3))
    nc.finalize()
    return nc


def _run(X, Y, log_length_scale, trace=False):
    from concourse.bass_utils import run_bass_kernel_spmd

    Lst, Rst = _prepare_inputs(X, Y, log_length_scale)
    if "nc" not in _cache:
        _cache["nc"] = _build_module()
    nc = _cache["nc"]
    in_maps = [
        {
            "lhsT": np.ascontiguousarray(Lst[:, c * NPC:(c + 1) * NPC]),
            "r_pl": Rst,
        }
        for c in range(NCORES)
    ]
    res = run_bass_kernel_spmd(nc, in_maps, core_ids=list(range(NCORES)),
                               trace=trace)
    # reassemble: per core out [3, 512, 4096] fp16 planes -> (1, 2N, 2M) f32
    planes = np.concatenate(
        [r["out"].reshape(3, NPC, M) for r in res.results], axis=1)  # (3, N, M)
    K = np.empty((N, 2, M, 2), dtype=np.float32)
    K[:, 0, :, 0] = planes[0]
    K[:, 0, :, 1] = planes[1]
    K[:, 1, :, 0] = planes[1]
    K[:, 1, :, 1] = planes[2]
    return K.reshape(1, 2 * N, 2 * M), res


def kernel(X, Y, log_length_scale):
    out, _ = _run(np.asarray(X), np.asarray(Y), np.asarray(log_length_scale))
    return out


# revision 4
# speedup vs baseline: 1.1491x; 1.0262x over previous
"""Divergence-free RBF kernel Gram matrix on 8 Trainium2 NeuronCores. v2.

Math: for d=2, with scaled coords x' = x*exp(-ll/2):
  dx = x0_i - y0_j, dy = x1_i - y1_j, r2 = dx^2 + dy^2, e = exp(-r2/2)
  K[2i+0, 2j+0] = e * (1 - dy^2)   = e * p00
  K[2i+0, 2j+1] = K[2i+1, 2j+0] = e * dx*dy = e * p01
  K[2i+1, 2j+1] = e * (1 - dx^2)   = e * p11

v2 design (vs v1): device emits three COMPACT fp16 planes out[c, i, j] =
e*p_c (c in {00, 01, 11}) instead of the interleaved f32 matrix; the host
interleaves rows/cols and upcasts. This cuts HBM writes 2.67x (12.6MB/core)
and halves+ the elementwise work. Each plane (and r2) is low-rank in the
X-basis {1, x0, x1, x0*x1, x0^2, x1^2} (K=6, hi/lo bf16 split stacked to
K=18 for fp32-grade accuracy): PE matmuls produce p01/p00/p11/r2 per
(128 x 512) chunk (PE col-rate 1.2GHz locked -> ~55us/core, the wall),
ACT computes e = exp(-r2/2) into fp16, and the 3 e*p multiplies per chunk
are split DVE-direct-from-PSUM vs GPSIMD-from-fp16-SBUF (after an ACT
copy) per a tunable class pattern so DVE/ACT/GP all stay under the PE wall.

Sharding: rows of X (n axis) split across 8 cores, 512 each; no comms.
"""

import numpy as np
import ml_dtypes

N = 4096          # X rows
M = 4096          # Y rows
D = 2
NCORES = 8
NPC = N // NCORES  # 512 X rows per core
IB = 128           # i-block = partition count
NIB = NPC // IB    # 4 i-blocks per core
JC = 512           # j-chunk per PSUM plane tile
NJC = M // JC      # 8 chunks per i-block
KST = 18           # stacked contraction dim (3 x 6 basis rows)

_cache = {}


def _hi_lo(a):
    bf = ml_dtypes.bfloat16
    hi = a.astype(bf)
    lo = (a - hi.astype(np.float64)).astype(bf)
    return hi, lo


def _prepare_inputs(X, Y, log_length_scale):
    s = float(np.exp(-0.5 * np.float64(np.asarray(log_length_scale).reshape(-1)[0])))
    xs = np.asarray(X, dtype=np.float64).reshape(N, D) * s
    ys = np.asarray(Y, dtype=np.float64).reshape(M, D) * s
    x0, x1 = xs[:, 0], xs[:, 1]
    y0, y1 = ys[:, 0], ys[:, 1]
    one_n, zero_m, one_m = np.ones(N), np.zeros(M), np.ones(M)

    # X-side basis [6, N]: rows {1, x0, x1, x0*x1, x0^2, x1^2}
    L = np.stack([one_n, x0, x1, x0 * x1, x0 ** 2, x1 ** 2])

    # Y-side coefficient columns [6, M] per plane; PE quadrant order is
    # (p01, p00, p11, r2) -- one plane per 32-partition row-tile
    c_01 = np.stack([y0 * y1, -y1, -y0, one_m, zero_m, zero_m])
    c_00 = np.stack([1 - y1 ** 2, zero_m, 2 * y1, zero_m, zero_m, -one_m])
    c_11 = np.stack([1 - y0 ** 2, 2 * y0, zero_m, zero_m, -one_m, zero_m])
    c_r2 = np.stack([y0 ** 2 + y1 ** 2, -2 * y0, -2 * y1, zero_m, one_m, one_m])

    Lh, Ll = _hi_lo(L)
    Lst = np.concatenate([Lh, Ll, Lh], axis=0).astype(np.float64)  # (18, N)

    bf = ml_dtypes.bfloat16
    Wq = np.zeros((128, N), dtype=bf)
    Rq = np.zeros((128, M), dtype=bf)
    for q, c in enumerate((c_01, c_00, c_11, c_r2)):
        Wq[32 * q:32 * q + KST] = Lst.astype(bf)
        Rh, Rl = _hi_lo(c)
        Rq[32 * q:32 * q + KST] = np.concatenate([Rh, Rh, Rl], axis=0)
    return Wq, Rq


def _build_module(bass_cls=None, **bass_kw):
    from concourse import bacc, mybir
    import concourse.tile as tile

    bf16 = mybir.dt.bfloat16
    f16 = mybir.dt.float16
    f32 = mybir.dt.float32
    Exp = mybir.ActivationFunctionType.Exp

    if bass_cls is None:
        bass_cls = bacc.Bacc
    nc = bass_cls("TRN2", target_bir_lowering=False, debug=False,
                  enable_asserts=False, **bass_kw)
    lhsT_d = nc.dram_tensor("lhsT", [128, NPC], bf16, kind="ExternalInput")
    r_d = nc.dram_tensor("r_pl", [128, M], bf16, kind="ExternalInput")
    # out planes: row-blocks (p00, p01, p11), each [NPC, M] fp16
    out_d = nc.dram_tensor("out", [3 * NPC, M], f16, kind="ExternalOutput")

    # psum plane slot (p01, p00, p11) -> output plane row-block
    OUT_SLOT = (1, 0, 2)
    JH = M // 2  # j-half per out tile / DMA

    with tile.TileContext(nc) as tc:
        with (
            tc.tile_pool(name="const", bufs=1) as cpool,
            tc.tile_pool(name="ep", bufs=4) as epool,
            tc.tile_pool(name="mp", bufs=4) as mpool,
            tc.tile_pool(name="outp", bufs=2) as opool,
            tc.tile_pool(name="r2ps", bufs=2, space="PSUM") as r2pool,
            tc.tile_pool(name="plAps", bufs=2, space="PSUM") as plApool,
            tc.tile_pool(name="plBps", bufs=2, space="PSUM") as plBpool,
        ):
            wt = cpool.tile([128, NPC], bf16)
            nc.sync.dma_start(out=wt[:], in_=lhsT_d[:, :])
            # quadrant-packed per-j-quarter input tiles (all 4 planes live in
            # one [128, MQ] tile, one plane per 32-partition quadrant)
            MQ = M // 4
            r_sb = [None] * 4
            for h in range(4):
                rt = cpool.tile([128, MQ], bf16, tag=f"rh{h}", name=f"rh{h}")
                nc.sync.dma_start(out=rt[:], in_=r_d[:, h * MQ:(h + 1) * MQ])
                r_sb[h] = rt

            out_v = out_d.ap().rearrange("(c i) j -> c i j", c=3)

            for ib in range(NIB):
                wtb = wt[:, ib * IB:(ib + 1) * IB]
                i0 = ib * IB
                od = out_v[:, i0:i0 + IB, :].rearrange("c p j -> p c j")
                # the very last half-block uses two quarter tiles so its DMA
                # starts halfway through and the drain tail is shorter
                segs = ([(0, 2), (2, 4)] if ib == NIB - 1 else [(0, 4)])
                for jq in range(2):
                  for (h0, h1) in (segs if jq == 1 else [(0, 4)]):
                    segw = (h1 - h0) * JC
                    out_sb = opool.tile([IB, 3 * segw], f16,
                                        tag=f"out{h1 - h0}", name="out_sb")
                    vout = out_sb[:].rearrange("p (c j) -> p c j", c=3)
                    for hc in range(h0, h1):
                        jc = jq * 4 + hc
                        rh, ro = jc // 2, (jc % 2) * JC
                        sc = hc - h0
                        # all 4 plane matmuls run CONCURRENTLY as 32x128 PE
                        # row-tiles (K=18 fits a 32-row quadrant): r2 on
                        # quadrant 3, p01/p00/p11 on quadrants 0/1/2
                        r2t = r2pool.tile([IB, JC], f32, tag="r2")
                        plA = plApool.tile([IB, 2 * JC], f32, tag="plA")
                        plB = plBpool.tile([IB, JC], f32, tag="plB")
                        rq = r_sb[rh]
                        dsts = (r2t[:], plA[:, 0:JC], plA[:, JC:2 * JC],
                                plB[:])
                        for q, dst in zip((3, 0, 1, 2), dsts):
                            nc.tensor.matmul(
                                dst,
                                wt[32 * q:32 * q + KST, i0:i0 + IB],
                                rq[32 * q:32 * q + KST, ro:ro + JC],
                                start=True, stop=True,
                                tile_position=(32 * q, 0))
                        et = epool.tile([IB, JC], f16, tag="e")
                        nc.scalar.activation(et[:], r2t[:], Exp, scale=-0.5)
                        mt = mpool.tile([IB, JC], f16, tag="m")
                        nc.scalar.copy(mt[:], plB[:])
                        esl = et[:]
                        for p in range(2):  # p01 -> plane 1, p00 -> plane 0
                            o = OUT_SLOT[p]
                            nc.vector.tensor_mul(
                                vout[:, o:o + 1, sc * JC:(sc + 1) * JC].squeeze(1),
                                plA[:, p * JC:(p + 1) * JC], esl)
                        nc.gpsimd.tensor_mul(
                            vout[:, 2:3, sc * JC:(sc + 1) * JC].squeeze(1),
                            mt[:], esl)
                    # DMA this segment (usually a half-block [128, 3, 2048]
                    # = 1.5 MB), partition-major on both sides so descriptors
                    # spray across all 16 SDMA engines
                    nc.sync.dma_start(
                        out=od[:, :, jq * JH + h0 * JC:jq * JH + h1 * JC],
                        in_=out_sb[:].rearrange("p (c j) -> p c j", c=3))
    nc.finalize()
    return nc


def _run(X, Y, log_length_scale, trace=False):
    from concourse.bass_utils import run_bass_kernel_spmd

    Wq, Rq = _prepare_inputs(X, Y, log_length_scale)
    if "nc" not in _cache:
        _cache["nc"] = _build_module()
    nc = _cache["nc"]
    in_maps = [
        {
            "lhsT": np.ascontiguousarray(Wq[:, c * NPC:(c + 1) * NPC]),
            "r_pl": Rq,
        }
        for c in range(NCORES)
    ]
    res = run_bass_kernel_spmd(nc, in_maps, core_ids=list(range(NCORES)),
                               trace=trace)
    # reassemble: per core out [3, 512, 4096] fp16 planes -> (1, 2N, 2M) f32
    planes = np.concatenate(
        [r["out"].reshape(3, NPC, M) for r in res.results], axis=1)  # (3, N, M)
    K = np.empty((N, 2, M, 2), dtype=np.float32)
    K[:, 0, :, 0] = planes[0]
    K[:, 0, :, 1] = planes[1]
    K[:, 1, :, 0] = planes[1]
    K[:, 1, :, 1] = planes[2]
    return K.reshape(1, 2 * N, 2 * M), res


def kernel(X, Y, log_length_scale):
    out, _ = _run(np.asarray(X), np.asarray(Y), np.asarray(log_length_scale))
    return out
